# revision 51
# baseline (speedup 1.0000x reference)
"""Trainium2 Bass kernel for T5-style relative-position-bias attention.

Problem (hardcoded): B=2, N=2048, H=16, D=64, MODEL=1024
  sim  = q @ k^T per head; sim = (sim + rel_pos_bias) * D**-0.5
  attn = softmax(sim, axis=-1)
  out  = (attn @ v) reshaped to [b, n, MODEL] @ w_out.T + b_out

Sharding: 8 cores = (batch b) x (query-chunk qoff in {0,512,1024,1536}).
Each core computes the full output rows for its 512 queries; no collectives.

Device algorithm per core (transposes all pushed to host):
  S^T[k, q] = kT_h-slices.T @ qT_h   (bf16 matmuls, contraction d=64).
  Host pre-scales q by LAM = 128*log2(e)*D**-0.5 so PSUM logits arrive in
  "bf16 exponent bits" scale: S = LAM*(q.k).
  Softmax exp is split across TWO engines to beat the single-ACT pace
  (ACT alone paces the pipeline at ~17.3us per head-pair):
   - tiles 0-4,6,8,10 of each head-pair: ACT Exp (scale=ln2/128).
   - tiles 5,7,9: ONE Vector-engine scalar_tensor_tensor per tile -
     int16(S + table) is the Schraudolph bf16-bit-pattern of exp
     (~1.8% rms on those keys only; softmax-averaged ~1.2% output error
     vs the 2e-2 gate).  Tiles 7/9 are all-band: their int16 table also
     carries the exact T5 bias (round(LAM*bias + BBITS)); tile 5 is
     const-only (tensor_scalar, bias folded into V').
    Positions 5,7,9 make the DVE stream chain-affine in the 2-buffer
    PSUM rotation: QK(7) waits ew(5)=DVE, QK(9) waits ew(7)=DVE, so the
    two engines stream their own dependency chains concurrently.
  T5 buckets saturate for |k-q| >= 128: chunks fully off the band have a
  CONSTANT bias factor per head, folded into host-pre-scaled V' tiles.
  Band chunks on ACT tiles (units 20,24,25,26,30,31) instead multiply by
  a bf16 exp(C*bias) table on the Vector engine.
  Key chunks are host-permuted into a fixed slot order (const slots 0..9,
  band slots 10..15) so all 8 cores run one identical program.
  No max-subtraction: logits are ~N(0,1) after scaling (safe in fp32).
  O^T[m, q] = sum_k V'[k, m] * P[k, q], V' = [V | ones] (row 64 = denom r).
  Normalize via broadcast 1/r, then y^T = w_out @ O^T + b_out (host
  re-transposes the per-core [1024, 512] outputs).
  Dummy warmup matmuls + a dummy Exp during the initial DMA window
  pre-warm the HAM clock gate and pre-load the ACT exp table.
"""
import sys
import math

sys.path.insert(0, "/opt/trn_rl_repo")

import numpy as np
import ml_dtypes

import concourse.bass as bass
from concourse import bacc
import concourse.tile as tile
from concourse import mybir
from concourse.bass_utils import run_bass_kernel_spmd

F32 = mybir.dt.float32
I16 = mybir.dt.int16
BF16 = mybir.dt.bfloat16

B, N, H, D = 2, 2048, 16, 64
MODEL = H * D
NQ = 512
NCORES = 8
C = float(D) ** -0.5
NUM_BUCKETS, MAX_DIST = 32, 128
CHUNKS = 16
NBAND = 6                       # band slots 10..15
LAM = 128.0 * C * math.log2(math.e)   # logits -> bf16-bit scale
BBITS = 16248.75                # Schraudolph bias (HW rounds f32->i16)
EXP_SCALE = math.log(2.0) / 128.0     # exp(S*EXP_SCALE) == exp(C*s)

DVE_TILES = (5, 7, 9)           # tiles exp'd on the Vector engine
# unit u = 2*slot + h01; tiles of 3 units; band units are 20..31
BTU_UNITS = (21, 22, 23, 27, 28, 29)  # DVE-side band units (int16 bias table)
BTM_UNITS = (20, 24, 25, 26, 30, 31)  # ACT-side band units (bf16 factor table)

_CACHE = {}


def _slot_map(qoff):
    """Permutation slot -> original key-chunk j. Band chunks (those touching
    |k-q| < 128 for q in [qoff, qoff+512)) go to slots 10..15; if fewer than
    6 band chunks exist, nearest const chunks fill the extra band slots."""
    j_lo = max(0, -(-(qoff - 254) // 128))          # ceil((qoff-254)/128)
    j_hi = min(CHUNKS - 1, (qoff + 638) // 128)     # floor
    band = list(range(j_lo, j_hi + 1))
    while len(band) < NBAND:                        # pad with neighbors
        if band[0] > 0:
            band.insert(0, band[0] - 1)
        else:
            band.append(band[-1] + 1)
    const = [j for j in range(CHUNKS) if j not in band]
    assert len(band) == NBAND and len(const) == CHUNKS - NBAND
    return const + band                             # slots 0..9 const, 10..15 band


def _build_bass():
    nc = bacc.Bacc("TRN2", target_bir_lowering=False, debug=False,
                   num_devices=NCORES)
    qt_d = nc.dram_tensor("qt", [8, 128, NQ], BF16, kind="ExternalInput")
    kt_d = nc.dram_tensor("kt", [8, 128, N], BF16, kind="ExternalInput")
    vv_d = nc.dram_tensor("vv", [H, 128, CHUNKS, D + 1], BF16, kind="ExternalInput")
    btu_d = nc.dram_tensor("btu", [8, 128, 6 * NQ], I16, kind="ExternalInput")
    btm_d = nc.dram_tensor("btm", [8, 128, 6 * NQ], BF16, kind="ExternalInput")
    wt_d = nc.dram_tensor("wt", [8, 128, MODEL], BF16, kind="ExternalInput")
    bv_d = nc.dram_tensor("bv", [128, 8], F32, kind="ExternalInput")
    yt_d = nc.dram_tensor("yt", [8, 128, NQ], F32, kind="ExternalOutput")

    with tile.TileContext(nc) as tc:
        with tc.tile_pool(name="const", bufs=1) as cpool:
            qt_ts = []
            for hp in range(8):
                t = cpool.tile([128, NQ], BF16, tag=f"qt{hp}", name=f"qt{hp}")
                qt_ts.append(t)
            nc.sync.dma_start(qt_ts[0], qt_d[0])
            # Warmup during the input-DMA window: dummy matmuls pre-warm the
            # HAM clock gate; a dummy Exp pre-loads the ACT spline table.
            wrm = cpool.tile([128, 128], BF16, tag="wrm", name="wrm")
            nc.vector.memset(wrm, 0.0)
            wrm_e = cpool.tile([1, 64], F32, tag="wrme", name="wrme")
            nc.scalar.activation(wrm_e, wrm[0:1, :64],
                                 mybir.ActivationFunctionType.Exp,
                                 bias=0.0, scale=0.0)
            with tc.tile_pool(name="wrmps", bufs=1, space="PSUM") as wpsp:
                wps = wpsp.tile([128, 128], F32, tag="wps", name="wps")
                for _ in range(45):
                    nc.tensor.matmul(wps, wrm, wrm, start=True, stop=True)
            wt_ts = []
            bv_t = None
            ocat_ts = [cpool.tile([128, NQ], BF16, tag=f"ocat{mc}",
                                  name=f"ocat{mc}")
                       for mc in range(8)]

            with tc.tile_pool(name="kt", bufs=2) as ktpool, \
                 tc.tile_pool(name="vv", bufs=4) as vvpool, \
                 tc.tile_pool(name="btu", bufs=2) as btupool, \
                 tc.tile_pool(name="btm", bufs=2) as btmpool, \
                 tc.tile_pool(name="p0", bufs=8) as p0pool, \
                 tc.tile_pool(name="pm", bufs=12) as pmpool, \
                 tc.tile_pool(name="sm", bufs=6) as smpool, \
                 tc.tile_pool(name="stps", bufs=2, space="PSUM") as stp, \
                 tc.tile_pool(name="ops", bufs=2, space="PSUM") as opool:
                DELAY = 4
                units_per_pair = [(s, h01) for s in range(CHUNKS)
                                  for h01 in range(2)]
                tiles = []
                for hp in range(8):
                    for ti, u0 in enumerate(range(0, 32, 3)):
                        tiles.append((hp, ti, units_per_pair[u0:u0 + 3],
                                      u0 == 0, u0 + 3 >= 32))
                state = {}      # hp -> (kt_t, vv_ts, btu_t, btm_t, o_pss)
                pv_queue = []   # (hp, chunk, srcs)
                tile_count = 0

                def emit_pv(hp, chunk, srcs):
                    _, vv_ts, _, _, o_pss = state[hp]
                    for uu, (s, h01) in enumerate(chunk):
                        nc.tensor.matmul(
                            o_pss[h01], vv_ts[h01][:, s, :], srcs[uu],
                            start=(s == 0), stop=(s == CHUNKS - 1))

                def emit_normalize(hp):
                    o_pss = state[hp][4]
                    for h01 in range(2):
                        o_ps = o_pss[h01]
                        rstage = smpool.tile([1, NQ], F32, tag="rstage",
                                             name="rstage")
                        nc.vector.tensor_copy(rstage[0:1, :], o_ps[64:65, :])
                        ocopy = smpool.tile([64, NQ], F32, tag="ocopy",
                                            name="ocopy")
                        nc.vector.tensor_copy(ocopy, o_ps[0:64, :])
                        rbr = smpool.tile([128, NQ], F32, tag="rbr",
                                          name="rbr")
                        nc.gpsimd.partition_broadcast(rbr, rstage)
                        rb = smpool.tile([128, NQ], F32, tag="rb", name="rb")
                        nc.vector.reciprocal_approx_fast(rb, rbr)
                        if h01 == 0:
                            nc.vector.tensor_tensor(
                                ocat_ts[hp][0:64, :], ocopy,
                                rb[0:64, :], mybir.AluOpType.mult)
                        else:
                            s64 = smpool.tile([64, NQ], BF16, tag="s64",
                                              name="s64")
                            nc.vector.tensor_tensor(
                                s64, ocopy, rb[0:64, :],
                                mybir.AluOpType.mult)
                            nc.sync.dma_start(ocat_ts[hp][64:128, :], s64)

                for hp, ti, chunk, is_first, is_last in tiles:
                    if is_first:
                        if hp == 6:
                            for mc in range(8):
                                t = cpool.tile([128, MODEL], BF16,
                                               tag=f"wt{mc}", name=f"wt{mc}")
                                nc.sync.dma_start(t, wt_d[mc])
                                wt_ts.append(t)
                            bv_t = cpool.tile([128, 8], F32, tag="bv",
                                              name="bv")
                            nc.sync.dma_start(bv_t, bv_d[:, :])
                        kt_t = ktpool.tile([128, N], BF16, tag="kt", name="kt")
                        if hp == 0:
                            nc.sync.dma_start(kt_t[:, :256], kt_d[hp][:, :256])
                            nc.sync.dma_start(kt_t[:, 256:], kt_d[hp][:, 256:])
                        else:
                            nc.sync.dma_start(kt_t, kt_d[hp])
                        if hp + 1 < 8:
                            nc.sync.dma_start(qt_ts[hp + 1], qt_d[hp + 1])
                        btu_t = btupool.tile([128, 6, NQ], I16, tag="btu",
                                             name="btu")
                        nc.sync.dma_start(
                            btu_t, btu_d[hp].rearrange("p (s f) -> p s f", s=6))
                        btm_t = btmpool.tile([128, 6, NQ], BF16, tag="btm",
                                             name="btm")
                        nc.sync.dma_start(
                            btm_t, btm_d[hp].rearrange("p (s f) -> p s f", s=6))
                        vv_ts, o_pss = [], []
                        for h01 in range(2):
                            h = 2 * hp + h01
                            vv_t = vvpool.tile([128, CHUNKS, D + 1], BF16,
                                               tag="vv", name="vv")
                            nc.sync.dma_start(vv_t, vv_d[h])
                            vv_ts.append(vv_t)
                            o_pss.append(opool.tile([D + 1, NQ], F32,
                                                    tag="ops", name="ops"))
                        state[hp] = (kt_t, vv_ts, btu_t, btm_t, o_pss)
                    kt_t, vv_ts, btu_t, btm_t, o_pss = state[hp]
                    nu = len(chunk)
                    gw = nu * NQ
                    st = stp.tile([128, 3 * NQ], F32, tag="st", name="st")
                    for uu, (s, h01) in enumerate(chunk):
                        lo, hi = h01 * 64, h01 * 64 + 64
                        nc.tensor.matmul(
                            st[:, uu * NQ:(uu + 1) * NQ],
                            kt_t[lo:hi, s * 128:(s + 1) * 128],
                            qt_ts[hp][lo:hi, :],
                            start=True, stop=True)
                    if (hp == 0 and ti in (0, 1, 2)) or \
                            (hp == 1 and ti in (0, 1)):
                        # fill the pipeline-fill PE gaps of the first head-
                        # pair so the HAM activity window stays busy; the
                        # garbage is overwritten by the first real PV's
                        # start=True.
                        for _ in range(3):
                            nc.tensor.matmul(
                                o_pss[0], wrm[0:64, 0:65],
                                qt_ts[hp][0:64, :],
                                start=True, stop=True)
                    u0 = 3 * ti
                    if ti in DVE_TILES:
                        ptile = pmpool.tile([128, 3 * NQ], BF16, tag="pm",
                                            name="pm")
                        if ti == 5:
                            # const-only tile: plain Schraudolph
                            nc.vector.tensor_scalar(
                                ptile[:, :gw].bitcast(I16), st[:, :gw],
                                1.0, BBITS,
                                mybir.AluOpType.mult, mybir.AluOpType.add)
                        else:
                            k0 = 0 if ti == 7 else 3
                            nc.vector.scalar_tensor_tensor(
                                ptile[:, :gw].bitcast(I16), st[:, :gw], 0.0,
                                btu_t[:, k0:k0 + nu, :].rearrange(
                                    "p a b -> p (a b)"),
                                mybir.AluOpType.add, mybir.AluOpType.add)
                        srcs = [ptile[:, uu * NQ:(uu + 1) * NQ]
                                for uu in range(nu)]
                    else:
                        ptile = p0pool.tile([128, 3 * NQ], BF16, tag="p0",
                                            name="p0")
                        nc.scalar.activation(
                            ptile[:, :gw], st[:, :gw],
                            mybir.ActivationFunctionType.Exp,
                            bias=0.0, scale=EXP_SCALE)
                        srcs = [ptile[:, uu * NQ:(uu + 1) * NQ]
                                for uu in range(nu)]
                        # band units on ACT tiles: multiply in the bf16
                        # exp(C*bias) factor; adjacent units share one op.
                        runs = []   # (uu0, n, btm_k0)
                        for uu, (s, h01) in enumerate(chunk):
                            u = u0 + uu
                            if u in BTM_UNITS:
                                k = BTM_UNITS.index(u)
                                if runs and runs[-1][0] + runs[-1][1] == uu \
                                        and runs[-1][2] + runs[-1][1] == k:
                                    runs[-1][1] += 1
                                else:
                                    runs.append([uu, 1, k])
                        for uu0, n, k in runs:
                            pm = pmpool.tile([128, 3 * NQ], BF16, tag="pm",
                                             name="pm")
                            nc.vector.tensor_tensor(
                                pm[:, :n * NQ],
                                ptile[:, uu0 * NQ:(uu0 + n) * NQ],
                                btm_t[:, k:k + n, :].rearrange(
                                    "p a b -> p (a b)"),
                                mybir.AluOpType.mult)
                            for j in range(n):
                                srcs[uu0 + j] = pm[:, j * NQ:(j + 1) * NQ]
                    pv_queue.append((hp, chunk, srcs, is_last))
                    # pop PV work in 2-tile batches: the PE pays ~120ns per
                    # QK<->PV weight/geometry switch, so fewer, larger blocks.
                    tile_count += 1
                    # pop after ODD tile indices: those boundaries fall on
                    # even unit counts, so the injected PV block never splits
                    # an h0/h1 QK row-pair (a split pair runs as two solo
                    # half-width windows: +216ns each).
                    if tile_count % 2 == 1:
                        while len(pv_queue) > DELAY:
                            qhp, qchunk, qsrcs, qlast = pv_queue.pop(0)
                            emit_pv(qhp, qchunk, qsrcs)
                            if qlast:
                                emit_normalize(qhp)
                for qhp, qchunk, qsrcs, qlast in pv_queue:
                    emit_pv(qhp, qchunk, qsrcs)
                    if qlast:
                        emit_normalize(qhp)

            with tc.tile_pool(name="ysb", bufs=2) as ypool, \
                 tc.tile_pool(name="fin", bufs=3, space="PSUM") as fpool:
                for ocp in range(4):
                    fp = fpool.tile([128, 2 * NQ], F32, tag="fp", name="fp")
                    for mc in range(8):
                        for half in range(2):
                            oc = 2 * ocp + half
                            nc.tensor.matmul(
                                fp[:, half * NQ:(half + 1) * NQ],
                                wt_ts[mc][:, oc * 128:(oc + 1) * 128],
                                ocat_ts[mc], start=(mc == 0), stop=(mc == 7))
                    ysb = ypool.tile([128, 2 * NQ], F32, tag="ysb", name="ysb")
                    for half in range(2):
                        oc = 2 * ocp + half
                        nc.scalar.add(ysb[:, half * NQ:(half + 1) * NQ],
                                      fp[:, half * NQ:(half + 1) * NQ],
                                      bv_t[:, oc:oc + 1])
                        nc.sync.dma_start(yt_d[oc],
                                          ysb[:, half * NQ:(half + 1) * NQ])
    nc.compile()
    return nc


def _rel_pos_bucket_np(rel):
    """T5 bidirectional bucketing, float32 math mirroring the jnp reference."""
    nb = NUM_BUCKETS // 2
    ret = (rel >= 0).astype(np.int32) * nb
    n = np.abs(rel)
    max_exact = nb // 2
    is_small = n < max_exact
    n_safe = np.maximum(n, 1).astype(np.float32)
    val_large = max_exact + (
        np.log(n_safe / np.float32(max_exact)).astype(np.float32)
        / np.float32(math.log(MAX_DIST / max_exact)) * np.float32(nb - max_exact)
    ).astype(np.int32)
    val_large = np.minimum(val_large, nb - 1)
    return ret + np.where(is_small, n, val_large)


def _tables(rel_emb):
    """Per-relative-offset tables: int16 Schraudolph bias bits and bf16
    multiplicative exp factors, both [H, 4095] for r in [-2047, 2047]."""
    rel = np.arange(-2047, 2048, dtype=np.int32)
    buckets = _rel_pos_bucket_np(rel)
    bias = np.asarray(rel_emb, np.float32)[buckets, :]          # [4095, H]
    bits = np.round(np.float32(LAM) * bias + np.float32(BBITS)).astype(np.int16)
    fac = np.exp(np.float32(C) * bias).astype(np.float32)
    return np.ascontiguousarray(bits.T), np.ascontiguousarray(fac.T)


def _prep_inputs(q, k, v, rel_emb, w_out, b_out):
    q = np.asarray(q, np.float32)
    k = np.asarray(k, np.float32)
    v = np.asarray(v, np.float32)
    rel_emb = np.asarray(rel_emb, np.float32)
    bits_diag, fac_diag = _tables(rel_emb)
    e_pos = np.exp(np.float32(C) * rel_emb[31, :])   # k - q >= 128
    e_neg = np.exp(np.float32(C) * rel_emb[15, :])   # k - q <= -128
    wt = np.ascontiguousarray(np.asarray(w_out, np.float32).T).reshape(8, 128, MODEL)
    bv = np.ascontiguousarray(np.asarray(b_out, np.float32).reshape(8, 128).T)
    p = np.arange(128)
    u = np.arange(NQ)
    in_maps = []
    for core in range(NCORES):
        b, qc = divmod(core, 4)
        qoff = qc * NQ
        smap = _slot_map(qoff)                       # slot -> chunk j
        qs = (q[b, qoff:qoff + NQ] * np.float32(LAM)).reshape(NQ, 8, 2, 64)
        qt = np.ascontiguousarray(qs.transpose(1, 2, 3, 0)).reshape(8, 128, NQ)
        kt = np.ascontiguousarray(
            k[b].reshape(N, 8, 2, 64).transpose(1, 2, 3, 0)).reshape(8, 128, N)
        kt = np.ascontiguousarray(
            kt.reshape(8, 128, CHUNKS, 128)[:, :, smap, :]).reshape(8, 128, N)
        vs = v[b].reshape(CHUNKS, 128, H, D).transpose(2, 1, 0, 3)  # [h,kk,j,d]
        vv = np.concatenate(
            [vs, np.ones((H, 128, CHUNKS, 1), np.float32)], axis=-1)
        vv = vv[:, :, smap, :]                       # slot order
        for s in range(CHUNKS - NBAND):
            j = smap[s]
            rel_min = 128 * j - qoff - (NQ - 1)      # min over tile of k - q
            rel_max = 128 * j + 127 - qoff
            if rel_min >= 128:
                fac = e_pos
            elif rel_max <= -128:
                fac = e_neg
            else:
                raise AssertionError(
                    f"band chunk {j} in const slot {s} (qoff={qoff})")
            vv[:, :, s, :] *= fac[:, None, None]
        # unit-ordered band tables (unit = 2*slot + h01)
        btu = np.empty((8, 128, 6, NQ), np.int16)
        btm = np.empty((8, 128, 6, NQ), np.float32)
        for arr, unit_list, diag in ((btu, BTU_UNITS, bits_diag),
                                     (btm, BTM_UNITS, fac_diag)):
            for kk, unit in enumerate(unit_list):
                slot, h01 = divmod(unit, 2)
                j = smap[slot]
                idx = (128 * j + p[:, None]) - (qoff + u[None, :]) + 2047
                for hp in range(8):
                    arr[hp, :, kk, :] = diag[2 * hp + h01][idx]
        in_maps.append({
            "qt": qt.astype(ml_dtypes.bfloat16),
            "kt": kt.astype(ml_dtypes.bfloat16),
            "vv": np.ascontiguousarray(vv).astype(ml_dtypes.bfloat16),
            "btu": np.ascontiguousarray(btu.reshape(8, 128, 6 * NQ)),
            "btm": np.ascontiguousarray(
                btm.reshape(8, 128, 6 * NQ).astype(ml_dtypes.bfloat16)),
            "wt": wt.astype(ml_dtypes.bfloat16), "bv": bv,
        })
    return in_maps


def _run(q, k, v, rel_emb, w_out, b_out, trace=False):
    if "nc" not in _CACHE:
        _CACHE["nc"] = _build_bass()
    nc = _CACHE["nc"]
    in_maps = _prep_inputs(q, k, v, rel_emb, w_out, b_out)
    res = run_bass_kernel_spmd(nc, in_maps, core_ids=list(range(NCORES)),
                               trace=trace)
    y = np.empty((B, N, MODEL), np.float32)
    for core in range(NCORES):
        b, qc = divmod(core, 4)
        qoff = qc * NQ
        yt = res.results[core]["yt"]
        y[b, qoff:qoff + NQ] = yt.transpose(2, 0, 1).reshape(NQ, MODEL)
    return y, res


def kernel(q, k, v, rel_emb, w_out, b_out):
    y, _ = _run(q, k, v, rel_emb, w_out, b_out, trace=False)
    return y


# revision 52
# speedup vs baseline: 1.0129x; 1.0129x over previous
"""Trainium2 Bass kernel for T5-style relative-position-bias attention.

Problem (hardcoded): B=2, N=2048, H=16, D=64, MODEL=1024
  sim  = q @ k^T per head; sim = (sim + rel_pos_bias) * D**-0.5
  attn = softmax(sim, axis=-1)
  out  = (attn @ v) reshaped to [b, n, MODEL] @ w_out.T + b_out

Sharding: 8 cores = (batch b) x (query-chunk qoff in {0,512,1024,1536}).
Each core computes the full output rows for its 512 queries; no collectives.

Device algorithm per core (transposes all pushed to host):
  S^T[k, q] = kT_h-slices.T @ qT_h   (bf16 matmuls, contraction d=64).
  Host pre-scales q by LAM = 128*log2(e)*D**-0.5 so PSUM logits arrive in
  "bf16 exponent bits" scale: S = LAM*(q.k).
  Softmax exp is split across TWO engines to beat the single-ACT pace
  (ACT alone paces the pipeline at ~17.3us per head-pair):
   - tiles 0-4,6,8,10 of each head-pair: ACT Exp (scale=ln2/128).
   - tiles 5,7,9: ONE Vector-engine scalar_tensor_tensor per tile -
     int16(S + table) is the Schraudolph bf16-bit-pattern of exp
     (~1.8% rms on those keys only; softmax-averaged ~1.2% output error
     vs the 2e-2 gate).  Tiles 7/9 are all-band: their int16 table also
     carries the exact T5 bias (round(LAM*bias + BBITS)); tile 5 is
     const-only (tensor_scalar, bias folded into V').
    Positions 5,7,9 make the DVE stream chain-affine in the 2-buffer
    PSUM rotation: QK(7) waits ew(5)=DVE, QK(9) waits ew(7)=DVE, so the
    two engines stream their own dependency chains concurrently.
  T5 buckets saturate for |k-q| >= 128: chunks fully off the band have a
  CONSTANT bias factor per head, folded into host-pre-scaled V' tiles.
  Band chunks on ACT tiles (units 20,24,25,26,30,31) instead multiply by
  a bf16 exp(C*bias) table on the Vector engine.
  Key chunks are host-permuted into a fixed slot order (const slots 0..9,
  band slots 10..15) so all 8 cores run one identical program.
  No max-subtraction: logits are ~N(0,1) after scaling (safe in fp32).
  O^T[m, q] = sum_k V'[k, m] * P[k, q], V' = [V | ones] (row 64 = denom r).
  Normalize via broadcast 1/r, then y^T = w_out @ O^T + b_out (host
  re-transposes the per-core [1024, 512] outputs).
  Dummy warmup matmuls + a dummy Exp during the initial DMA window
  pre-warm the HAM clock gate and pre-load the ACT exp table.
"""
import sys
import math

sys.path.insert(0, "/opt/trn_rl_repo")

import numpy as np
import ml_dtypes

import concourse.bass as bass
from concourse import bacc
import concourse.tile as tile
from concourse import mybir
from concourse.bass_utils import run_bass_kernel_spmd

F32 = mybir.dt.float32
I16 = mybir.dt.int16
BF16 = mybir.dt.bfloat16

B, N, H, D = 2, 2048, 16, 64
MODEL = H * D
NQ = 512
NCORES = 8
C = float(D) ** -0.5
NUM_BUCKETS, MAX_DIST = 32, 128
CHUNKS = 16
NBAND = 6                       # band slots 10..15
LAM = 128.0 * C * math.log2(math.e)   # logits -> bf16-bit scale
BBITS = 16248.75                # Schraudolph bias (HW rounds f32->i16)
EXP_SCALE = math.log(2.0) / 128.0     # exp(S*EXP_SCALE) == exp(C*s)

DVE_TILES = (5, 7, 9)           # tiles exp'd on the Vector engine
# unit u = 2*slot + h01; tiles of 3 units; band units are 20..31
BTU_UNITS = (21, 22, 23, 27, 28, 29)  # DVE-side band units (int16 bias table)
BTM_UNITS = (20, 24, 25, 26, 30, 31)  # ACT-side band units (bf16 factor table)

_CACHE = {}


def _slot_map(qoff):
    """Permutation slot -> original key-chunk j. Band chunks (those touching
    |k-q| < 128 for q in [qoff, qoff+512)) go to slots 10..15; if fewer than
    6 band chunks exist, nearest const chunks fill the extra band slots."""
    j_lo = max(0, -(-(qoff - 254) // 128))          # ceil((qoff-254)/128)
    j_hi = min(CHUNKS - 1, (qoff + 638) // 128)     # floor
    band = list(range(j_lo, j_hi + 1))
    while len(band) < NBAND:                        # pad with neighbors
        if band[0] > 0:
            band.insert(0, band[0] - 1)
        else:
            band.append(band[-1] + 1)
    const = [j for j in range(CHUNKS) if j not in band]
    assert len(band) == NBAND and len(const) == CHUNKS - NBAND
    return const + band                             # slots 0..9 const, 10..15 band


def _build_bass():
    nc = bacc.Bacc("TRN2", target_bir_lowering=False, debug=False,
                   num_devices=NCORES)
    qt_d = nc.dram_tensor("qt", [8, 128, NQ], BF16, kind="ExternalInput")
    kt_d = nc.dram_tensor("kt", [8, 128, N], BF16, kind="ExternalInput")
    vv_d = nc.dram_tensor("vv", [H, 128, CHUNKS, D + 1], BF16, kind="ExternalInput")
    btu_d = nc.dram_tensor("btu", [8, 128, 6 * NQ], I16, kind="ExternalInput")
    btm_d = nc.dram_tensor("btm", [8, 128, 6 * NQ], BF16, kind="ExternalInput")
    wt_d = nc.dram_tensor("wt", [8, 128, MODEL], BF16, kind="ExternalInput")
    bv_d = nc.dram_tensor("bv", [128, 8], F32, kind="ExternalInput")
    yt_d = nc.dram_tensor("yt", [8, 128, NQ], F32, kind="ExternalOutput")

    with tile.TileContext(nc) as tc:
        with tc.tile_pool(name="const", bufs=1) as cpool:
            qt_ts = []
            for hp in range(8):
                t = cpool.tile([128, NQ], BF16, tag=f"qt{hp}", name=f"qt{hp}")
                qt_ts.append(t)
            nc.sync.dma_start(qt_ts[0], qt_d[0])
            # Warmup during the input-DMA window: dummy matmuls pre-warm the
            # HAM clock gate; a dummy Exp pre-loads the ACT spline table.
            wrm = cpool.tile([128, 128], BF16, tag="wrm", name="wrm")
            nc.vector.memset(wrm, 0.0)
            wrm_e = cpool.tile([1, 64], F32, tag="wrme", name="wrme")
            nc.scalar.activation(wrm_e, wrm[0:1, :64],
                                 mybir.ActivationFunctionType.Exp,
                                 bias=0.0, scale=0.0)
            with tc.tile_pool(name="wrmps", bufs=1, space="PSUM") as wpsp:
                wps = wpsp.tile([128, 128], F32, tag="wps", name="wps")
                for _ in range(45):
                    nc.tensor.matmul(wps, wrm, wrm, start=True, stop=True)
            wt_ts = []
            bv_t = None
            ocat_ts = [cpool.tile([128, NQ], BF16, tag=f"ocat{mc}",
                                  name=f"ocat{mc}")
                       for mc in range(8)]

            with tc.tile_pool(name="kt", bufs=2) as ktpool, \
                 tc.tile_pool(name="vv", bufs=4) as vvpool, \
                 tc.tile_pool(name="btu", bufs=2) as btupool, \
                 tc.tile_pool(name="btm", bufs=2) as btmpool, \
                 tc.tile_pool(name="p0", bufs=8) as p0pool, \
                 tc.tile_pool(name="pm", bufs=12) as pmpool, \
                 tc.tile_pool(name="sm", bufs=6) as smpool, \
                 tc.tile_pool(name="stps", bufs=2, space="PSUM") as stp, \
                 tc.tile_pool(name="ops", bufs=2, space="PSUM") as opool:
                DELAY = 4
                units_per_pair = [(s, h01) for s in range(CHUNKS)
                                  for h01 in range(2)]
                tiles = []
                for hp in range(8):
                    for ti, u0 in enumerate(range(0, 32, 3)):
                        tiles.append((hp, ti, units_per_pair[u0:u0 + 3],
                                      u0 == 0, u0 + 3 >= 32))
                state = {}      # hp -> (kt_t, vv_ts, btu_t, btm_t, o_pss)
                pv_queue = []   # (hp, chunk, srcs)
                tile_count = 0

                def emit_pv(hp, chunk, srcs):
                    _, vv_ts, _, _, o_pss = state[hp]
                    for uu, (s, h01) in enumerate(chunk):
                        nc.tensor.matmul(
                            o_pss[h01], vv_ts[h01][:, s, :], srcs[uu],
                            start=(s == 0), stop=(s == CHUNKS - 1))

                def emit_normalize(hp):
                    o_pss = state[hp][4]
                    for h01 in range(2):
                        o_ps = o_pss[h01]
                        rstage = smpool.tile([1, NQ], F32, tag="rstage",
                                             name="rstage")
                        nc.vector.tensor_copy(rstage[0:1, :], o_ps[64:65, :])
                        ocopy = smpool.tile([64, NQ], F32, tag="ocopy",
                                            name="ocopy")
                        nc.vector.tensor_copy(ocopy, o_ps[0:64, :])
                        rbr = smpool.tile([128, NQ], F32, tag="rbr",
                                          name="rbr")
                        nc.gpsimd.partition_broadcast(rbr, rstage)
                        rb = smpool.tile([128, NQ], F32, tag="rb", name="rb")
                        nc.vector.reciprocal_approx_fast(rb, rbr)
                        if h01 == 0:
                            nc.vector.tensor_tensor(
                                ocat_ts[hp][0:64, :], ocopy,
                                rb[0:64, :], mybir.AluOpType.mult)
                        else:
                            s64 = smpool.tile([64, NQ], BF16, tag="s64",
                                              name="s64")
                            nc.vector.tensor_tensor(
                                s64, ocopy, rb[0:64, :],
                                mybir.AluOpType.mult)
                            nc.sync.dma_start(ocat_ts[hp][64:128, :], s64)

                for hp, ti, chunk, is_first, is_last in tiles:
                    if is_first:
                        if hp == 6:
                            for mc in range(8):
                                t = cpool.tile([128, MODEL], BF16,
                                               tag=f"wt{mc}", name=f"wt{mc}")
                                nc.sync.dma_start(t, wt_d[mc])
                                wt_ts.append(t)
                            bv_t = cpool.tile([128, 8], F32, tag="bv",
                                              name="bv")
                            nc.sync.dma_start(bv_t, bv_d[:, :])
                        kt_t = ktpool.tile([128, N], BF16, tag="kt", name="kt")
                        if hp == 0:
                            nc.sync.dma_start(kt_t[:, :256], kt_d[hp][:, :256])
                            nc.sync.dma_start(kt_t[:, 256:], kt_d[hp][:, 256:])
                        else:
                            nc.sync.dma_start(kt_t, kt_d[hp])
                        if hp + 1 < 8:
                            nc.sync.dma_start(qt_ts[hp + 1], qt_d[hp + 1])
                        btu_t = btupool.tile([128, 6, NQ], I16, tag="btu",
                                             name="btu")
                        nc.sync.dma_start(
                            btu_t, btu_d[hp].rearrange("p (s f) -> p s f", s=6))
                        btm_t = btmpool.tile([128, 6, NQ], BF16, tag="btm",
                                             name="btm")
                        nc.sync.dma_start(
                            btm_t, btm_d[hp].rearrange("p (s f) -> p s f", s=6))
                        vv_ts, o_pss = [], []
                        for h01 in range(2):
                            h = 2 * hp + h01
                            vv_t = vvpool.tile([128, CHUNKS, D + 1], BF16,
                                               tag="vv", name="vv")
                            nc.sync.dma_start(vv_t, vv_d[h])
                            vv_ts.append(vv_t)
                            o_pss.append(opool.tile([D + 1, NQ], F32,
                                                    tag="ops", name="ops"))
                        state[hp] = (kt_t, vv_ts, btu_t, btm_t, o_pss)
                    kt_t, vv_ts, btu_t, btm_t, o_pss = state[hp]
                    nu = len(chunk)
                    gw = nu * NQ
                    st = stp.tile([128, 3 * NQ], F32, tag="st", name="st")
                    for uu, (s, h01) in enumerate(chunk):
                        lo, hi = h01 * 64, h01 * 64 + 64
                        nc.tensor.matmul(
                            st[:, uu * NQ:(uu + 1) * NQ],
                            kt_t[lo:hi, s * 128:(s + 1) * 128],
                            qt_ts[hp][lo:hi, :],
                            start=True, stop=True)
                    if hp == 0 and ti in (0, 1, 2):
                        # fill the pipeline-fill PE gaps of the first head-
                        # pair so the HAM activity window stays busy; the
                        # garbage is overwritten by the first real PV's
                        # start=True.
                        for _ in range(3):
                            nc.tensor.matmul(
                                o_pss[0], wrm[0:64, 0:65],
                                qt_ts[hp][0:64, :],
                                start=True, stop=True)
                    u0 = 3 * ti
                    if ti in DVE_TILES:
                        ptile = pmpool.tile([128, 3 * NQ], BF16, tag="pm",
                                            name="pm")
                        if ti == 5:
                            # const-only tile: plain Schraudolph
                            nc.vector.tensor_scalar(
                                ptile[:, :gw].bitcast(I16), st[:, :gw],
                                1.0, BBITS,
                                mybir.AluOpType.mult, mybir.AluOpType.add)
                        else:
                            k0 = 0 if ti == 7 else 3
                            nc.vector.scalar_tensor_tensor(
                                ptile[:, :gw].bitcast(I16), st[:, :gw], 0.0,
                                btu_t[:, k0:k0 + nu, :].rearrange(
                                    "p a b -> p (a b)"),
                                mybir.AluOpType.add, mybir.AluOpType.add)
                        srcs = [ptile[:, uu * NQ:(uu + 1) * NQ]
                                for uu in range(nu)]
                    else:
                        ptile = p0pool.tile([128, 3 * NQ], BF16, tag="p0",
                                            name="p0")
                        nc.scalar.activation(
                            ptile[:, :gw], st[:, :gw],
                            mybir.ActivationFunctionType.Exp,
                            bias=0.0, scale=EXP_SCALE)
                        srcs = [ptile[:, uu * NQ:(uu + 1) * NQ]
                                for uu in range(nu)]
                        # band units on ACT tiles: multiply in the bf16
                        # exp(C*bias) factor; adjacent units share one op.
                        runs = []   # (uu0, n, btm_k0)
                        for uu, (s, h01) in enumerate(chunk):
                            u = u0 + uu
                            if u in BTM_UNITS:
                                k = BTM_UNITS.index(u)
                                if runs and runs[-1][0] + runs[-1][1] == uu \
                                        and runs[-1][2] + runs[-1][1] == k:
                                    runs[-1][1] += 1
                                else:
                                    runs.append([uu, 1, k])
                        for uu0, n, k in runs:
                            pm = pmpool.tile([128, 3 * NQ], BF16, tag="pm",
                                             name="pm")
                            nc.vector.tensor_tensor(
                                pm[:, :n * NQ],
                                ptile[:, uu0 * NQ:(uu0 + n) * NQ],
                                btm_t[:, k:k + n, :].rearrange(
                                    "p a b -> p (a b)"),
                                mybir.AluOpType.mult)
                            for j in range(n):
                                srcs[uu0 + j] = pm[:, j * NQ:(j + 1) * NQ]
                    pv_queue.append((hp, chunk, srcs, is_last))
                    # pop PV work in 2-tile batches: the PE pays ~120ns per
                    # QK<->PV weight/geometry switch, so fewer, larger blocks.
                    tile_count += 1
                    # pop after ODD tile indices: those boundaries fall on
                    # even unit counts, so the injected PV block never splits
                    # an h0/h1 QK row-pair (a split pair runs as two solo
                    # half-width windows: +216ns each).
                    if tile_count % 2 == 1:
                        while len(pv_queue) > DELAY:
                            qhp, qchunk, qsrcs, qlast = pv_queue.pop(0)
                            emit_pv(qhp, qchunk, qsrcs)
                            if qlast:
                                emit_normalize(qhp)
                for qhp, qchunk, qsrcs, qlast in pv_queue:
                    emit_pv(qhp, qchunk, qsrcs)
                    if qlast:
                        emit_normalize(qhp)

            with tc.tile_pool(name="ysb", bufs=2) as ypool, \
                 tc.tile_pool(name="fin", bufs=3, space="PSUM") as fpool:
                for ocp in range(4):
                    fp = fpool.tile([128, 2 * NQ], F32, tag="fp", name="fp")
                    for mc in range(8):
                        for half in range(2):
                            oc = 2 * ocp + half
                            nc.tensor.matmul(
                                fp[:, half * NQ:(half + 1) * NQ],
                                wt_ts[mc][:, oc * 128:(oc + 1) * 128],
                                ocat_ts[mc], start=(mc == 0), stop=(mc == 7))
                    ysb = ypool.tile([128, 2 * NQ], F32, tag="ysb", name="ysb")
                    for half in range(2):
                        oc = 2 * ocp + half
                        nc.scalar.add(ysb[:, half * NQ:(half + 1) * NQ],
                                      fp[:, half * NQ:(half + 1) * NQ],
                                      bv_t[:, oc:oc + 1])
                        nc.sync.dma_start(yt_d[oc],
                                          ysb[:, half * NQ:(half + 1) * NQ])
    nc.compile()
    return nc


def _rel_pos_bucket_np(rel):
    """T5 bidirectional bucketing, float32 math mirroring the jnp reference."""
    nb = NUM_BUCKETS // 2
    ret = (rel >= 0).astype(np.int32) * nb
    n = np.abs(rel)
    max_exact = nb // 2
    is_small = n < max_exact
    n_safe = np.maximum(n, 1).astype(np.float32)
    val_large = max_exact + (
        np.log(n_safe / np.float32(max_exact)).astype(np.float32)
        / np.float32(math.log(MAX_DIST / max_exact)) * np.float32(nb - max_exact)
    ).astype(np.int32)
    val_large = np.minimum(val_large, nb - 1)
    return ret + np.where(is_small, n, val_large)


def _tables(rel_emb):
    """Per-relative-offset tables: int16 Schraudolph bias bits and bf16
    multiplicative exp factors, both [H, 4095] for r in [-2047, 2047]."""
    rel = np.arange(-2047, 2048, dtype=np.int32)
    buckets = _rel_pos_bucket_np(rel)
    bias = np.asarray(rel_emb, np.float32)[buckets, :]          # [4095, H]
    bits = np.round(np.float32(LAM) * bias + np.float32(BBITS)).astype(np.int16)
    fac = np.exp(np.float32(C) * bias).astype(np.float32)
    return np.ascontiguousarray(bits.T), np.ascontiguousarray(fac.T)


def _prep_inputs(q, k, v, rel_emb, w_out, b_out):
    q = np.asarray(q, np.float32)
    k = np.asarray(k, np.float32)
    v = np.asarray(v, np.float32)
    rel_emb = np.asarray(rel_emb, np.float32)
    bits_diag, fac_diag = _tables(rel_emb)
    e_pos = np.exp(np.float32(C) * rel_emb[31, :])   # k - q >= 128
    e_neg = np.exp(np.float32(C) * rel_emb[15, :])   # k - q <= -128
    wt = np.ascontiguousarray(np.asarray(w_out, np.float32).T).reshape(8, 128, MODEL)
    bv = np.ascontiguousarray(np.asarray(b_out, np.float32).reshape(8, 128).T)
    p = np.arange(128)
    u = np.arange(NQ)
    in_maps = []
    for core in range(NCORES):
        b, qc = divmod(core, 4)
        qoff = qc * NQ
        smap = _slot_map(qoff)                       # slot -> chunk j
        qs = (q[b, qoff:qoff + NQ] * np.float32(LAM)).reshape(NQ, 8, 2, 64)
        qt = np.ascontiguousarray(qs.transpose(1, 2, 3, 0)).reshape(8, 128, NQ)
        kt = np.ascontiguousarray(
            k[b].reshape(N, 8, 2, 64).transpose(1, 2, 3, 0)).reshape(8, 128, N)
        kt = np.ascontiguousarray(
            kt.reshape(8, 128, CHUNKS, 128)[:, :, smap, :]).reshape(8, 128, N)
        vs = v[b].reshape(CHUNKS, 128, H, D).transpose(2, 1, 0, 3)  # [h,kk,j,d]
        vv = np.concatenate(
            [vs, np.ones((H, 128, CHUNKS, 1), np.float32)], axis=-1)
        vv = vv[:, :, smap, :]                       # slot order
        for s in range(CHUNKS - NBAND):
            j = smap[s]
            rel_min = 128 * j - qoff - (NQ - 1)      # min over tile of k - q
            rel_max = 128 * j + 127 - qoff
            if rel_min >= 128:
                fac = e_pos
            elif rel_max <= -128:
                fac = e_neg
            else:
                raise AssertionError(
                    f"band chunk {j} in const slot {s} (qoff={qoff})")
            vv[:, :, s, :] *= fac[:, None, None]
        # unit-ordered band tables (unit = 2*slot + h01)
        btu = np.empty((8, 128, 6, NQ), np.int16)
        btm = np.empty((8, 128, 6, NQ), np.float32)
        for arr, unit_list, diag in ((btu, BTU_UNITS, bits_diag),
                                     (btm, BTM_UNITS, fac_diag)):
            for kk, unit in enumerate(unit_list):
                slot, h01 = divmod(unit, 2)
                j = smap[slot]
                idx = (128 * j + p[:, None]) - (qoff + u[None, :]) + 2047
                for hp in range(8):
                    arr[hp, :, kk, :] = diag[2 * hp + h01][idx]
        in_maps.append({
            "qt": qt.astype(ml_dtypes.bfloat16),
            "kt": kt.astype(ml_dtypes.bfloat16),
            "vv": np.ascontiguousarray(vv).astype(ml_dtypes.bfloat16),
            "btu": np.ascontiguousarray(btu.reshape(8, 128, 6 * NQ)),
            "btm": np.ascontiguousarray(
                btm.reshape(8, 128, 6 * NQ).astype(ml_dtypes.bfloat16)),
            "wt": wt.astype(ml_dtypes.bfloat16), "bv": bv,
        })
    return in_maps


def _run(q, k, v, rel_emb, w_out, b_out, trace=False):
    if "nc" not in _CACHE:
        _CACHE["nc"] = _build_bass()
    nc = _CACHE["nc"]
    in_maps = _prep_inputs(q, k, v, rel_emb, w_out, b_out)
    res = run_bass_kernel_spmd(nc, in_maps, core_ids=list(range(NCORES)),
                               trace=trace)
    y = np.empty((B, N, MODEL), np.float32)
    for core in range(NCORES):
        b, qc = divmod(core, 4)
        qoff = qc * NQ
        yt = res.results[core]["yt"]
        y[b, qoff:qoff + NQ] = yt.transpose(2, 0, 1).reshape(NQ, MODEL)
    return y, res


def kernel(q, k, v, rel_emb, w_out, b_out):
    y, _ = _run(q, k, v, rel_emb, w_out, b_out, trace=False)
    return y


# revision 53
# speedup vs baseline: 1.0194x; 1.0065x over previous
"""Trainium2 Bass kernel for T5-style relative-position-bias attention.

Problem (hardcoded): B=2, N=2048, H=16, D=64, MODEL=1024
  sim  = q @ k^T per head; sim = (sim + rel_pos_bias) * D**-0.5
  attn = softmax(sim, axis=-1)
  out  = (attn @ v) reshaped to [b, n, MODEL] @ w_out.T + b_out

Sharding: 8 cores = (batch b) x (query-chunk qoff in {0,512,1024,1536}).
Each core computes the full output rows for its 512 queries; no collectives.

Device algorithm per core (transposes all pushed to host):
  S^T[k, q] = kT_h-slices.T @ qT_h   (bf16 matmuls, contraction d=64).
  Host pre-scales q by LAM = 128*log2(e)*D**-0.5 so PSUM logits arrive in
  "bf16 exponent bits" scale: S = LAM*(q.k).
  Softmax exp is split across TWO engines to beat the single-ACT pace
  (ACT alone paces the pipeline at ~17.3us per head-pair):
   - tiles 0-4,6,8,10 of each head-pair: ACT Exp (scale=ln2/128).
   - tiles 5,7,9: ONE Vector-engine scalar_tensor_tensor per tile -
     int16(S + table) is the Schraudolph bf16-bit-pattern of exp
     (~1.8% rms on those keys only; softmax-averaged ~1.2% output error
     vs the 2e-2 gate).  Tiles 7/9 are all-band: their int16 table also
     carries the exact T5 bias (round(LAM*bias + BBITS)); tile 5 is
     const-only (tensor_scalar, bias folded into V').
    Positions 5,7,9 make the DVE stream chain-affine in the 2-buffer
    PSUM rotation: QK(7) waits ew(5)=DVE, QK(9) waits ew(7)=DVE, so the
    two engines stream their own dependency chains concurrently.
  T5 buckets saturate for |k-q| >= 128: chunks fully off the band have a
  CONSTANT bias factor per head, folded into host-pre-scaled V' tiles.
  Band chunks on ACT tiles (units 20,24,25,26,30,31) instead multiply by
  a bf16 exp(C*bias) table on the Vector engine.
  Key chunks are host-permuted into a fixed slot order (const slots 0..9,
  band slots 10..15) so all 8 cores run one identical program.
  No max-subtraction: logits are ~N(0,1) after scaling (safe in fp32).
  O^T[m, q] = sum_k V'[k, m] * P[k, q], V' = [V | ones] (row 64 = denom r).
  Normalize via broadcast 1/r, then y^T = w_out @ O^T + b_out (host
  re-transposes the per-core [1024, 512] outputs).
  Dummy warmup matmuls + a dummy Exp during the initial DMA window
  pre-warm the HAM clock gate and pre-load the ACT exp table.
"""
import sys
import math

sys.path.insert(0, "/opt/trn_rl_repo")

import numpy as np
import ml_dtypes

import concourse.bass as bass
from concourse import bacc
import concourse.tile as tile
from concourse import mybir
from concourse.bass_utils import run_bass_kernel_spmd

F32 = mybir.dt.float32
I16 = mybir.dt.int16
BF16 = mybir.dt.bfloat16

B, N, H, D = 2, 2048, 16, 64
MODEL = H * D
NQ = 512
NCORES = 8
C = float(D) ** -0.5
NUM_BUCKETS, MAX_DIST = 32, 128
CHUNKS = 16
NBAND = 6                       # band slots 10..15
LAM = 128.0 * C * math.log2(math.e)   # logits -> bf16-bit scale
BBITS = 16248.75                # Schraudolph bias (HW rounds f32->i16)
EXP_SCALE = math.log(2.0) / 128.0     # exp(S*EXP_SCALE) == exp(C*s)

DVE_TILES = (5, 7, 9)           # tiles exp'd on the Vector engine
# unit u = 2*slot + h01; tiles of 3 units; band units are 20..31
BTU_UNITS = (21, 22, 23, 27, 28, 29)  # DVE-side band units (int16 bias table)
BTM_UNITS = (20, 24, 25, 26, 30, 31)  # ACT-side band units (bf16 factor table)

_CACHE = {}


def _slot_map(qoff):
    """Permutation slot -> original key-chunk j. Band chunks (those touching
    |k-q| < 128 for q in [qoff, qoff+512)) go to slots 10..15; if fewer than
    6 band chunks exist, nearest const chunks fill the extra band slots."""
    j_lo = max(0, -(-(qoff - 254) // 128))          # ceil((qoff-254)/128)
    j_hi = min(CHUNKS - 1, (qoff + 638) // 128)     # floor
    band = list(range(j_lo, j_hi + 1))
    while len(band) < NBAND:                        # pad with neighbors
        if band[0] > 0:
            band.insert(0, band[0] - 1)
        else:
            band.append(band[-1] + 1)
    const = [j for j in range(CHUNKS) if j not in band]
    assert len(band) == NBAND and len(const) == CHUNKS - NBAND
    return const + band                             # slots 0..9 const, 10..15 band


def _build_bass():
    nc = bacc.Bacc("TRN2", target_bir_lowering=False, debug=False,
                   num_devices=NCORES)
    qt_d = nc.dram_tensor("qt", [8, 128, NQ], BF16, kind="ExternalInput")
    kt_d = nc.dram_tensor("kt", [8, 128, N], BF16, kind="ExternalInput")
    vv_d = nc.dram_tensor("vv", [H, 128, CHUNKS, D + 1], BF16, kind="ExternalInput")
    btu_d = nc.dram_tensor("btu", [8, 128, 6 * NQ], I16, kind="ExternalInput")
    btm_d = nc.dram_tensor("btm", [8, 128, 6 * NQ], BF16, kind="ExternalInput")
    wt_d = nc.dram_tensor("wt", [8, 128, MODEL], BF16, kind="ExternalInput")
    bv_d = nc.dram_tensor("bv", [128, 8], F32, kind="ExternalInput")
    yt_d = nc.dram_tensor("yt", [8, 128, NQ], F32, kind="ExternalOutput")

    with tile.TileContext(nc) as tc:
        with tc.tile_pool(name="const", bufs=1) as cpool:
            qt_ts = []
            for hp in range(8):
                t = cpool.tile([128, NQ], BF16, tag=f"qt{hp}", name=f"qt{hp}")
                qt_ts.append(t)
            nc.sync.dma_start(qt_ts[0], qt_d[0])
            # Warmup during the input-DMA window: dummy matmuls pre-warm the
            # HAM clock gate; a dummy Exp pre-loads the ACT spline table.
            wrm = cpool.tile([128, 128], BF16, tag="wrm", name="wrm")
            nc.vector.memset(wrm, 0.0)
            wrm_e = cpool.tile([1, 64], F32, tag="wrme", name="wrme")
            nc.scalar.activation(wrm_e, wrm[0:1, :64],
                                 mybir.ActivationFunctionType.Exp,
                                 bias=0.0, scale=0.0)
            with tc.tile_pool(name="wrmps", bufs=1, space="PSUM") as wpsp:
                wps = wpsp.tile([128, 128], F32, tag="wps", name="wps")
                for _ in range(30):
                    nc.tensor.matmul(wps, wrm, wrm, start=True, stop=True)
            wt_ts = []
            bv_t = None
            ocat_ts = [cpool.tile([128, NQ], BF16, tag=f"ocat{mc}",
                                  name=f"ocat{mc}")
                       for mc in range(8)]

            with tc.tile_pool(name="kt", bufs=2) as ktpool, \
                 tc.tile_pool(name="vv", bufs=4) as vvpool, \
                 tc.tile_pool(name="btu", bufs=2) as btupool, \
                 tc.tile_pool(name="btm", bufs=2) as btmpool, \
                 tc.tile_pool(name="p0", bufs=8) as p0pool, \
                 tc.tile_pool(name="pm", bufs=12) as pmpool, \
                 tc.tile_pool(name="sm", bufs=6) as smpool, \
                 tc.tile_pool(name="stps", bufs=2, space="PSUM") as stp, \
                 tc.tile_pool(name="ops", bufs=2, space="PSUM") as opool:
                DELAY = 4
                units_per_pair = [(s, h01) for s in range(CHUNKS)
                                  for h01 in range(2)]
                tiles = []
                for hp in range(8):
                    for ti, u0 in enumerate(range(0, 32, 3)):
                        tiles.append((hp, ti, units_per_pair[u0:u0 + 3],
                                      u0 == 0, u0 + 3 >= 32))
                state = {}      # hp -> (kt_t, vv_ts, btu_t, btm_t, o_pss)
                pv_queue = []   # (hp, chunk, srcs)
                tile_count = 0

                def emit_pv(hp, chunk, srcs):
                    _, vv_ts, _, _, o_pss = state[hp]
                    for uu, (s, h01) in enumerate(chunk):
                        nc.tensor.matmul(
                            o_pss[h01], vv_ts[h01][:, s, :], srcs[uu],
                            start=(s == 0), stop=(s == CHUNKS - 1))

                def emit_normalize(hp):
                    o_pss = state[hp][4]
                    for h01 in range(2):
                        o_ps = o_pss[h01]
                        rstage = smpool.tile([1, NQ], F32, tag="rstage",
                                             name="rstage")
                        nc.vector.tensor_copy(rstage[0:1, :], o_ps[64:65, :])
                        ocopy = smpool.tile([64, NQ], F32, tag="ocopy",
                                            name="ocopy")
                        nc.vector.tensor_copy(ocopy, o_ps[0:64, :])
                        rbr = smpool.tile([128, NQ], F32, tag="rbr",
                                          name="rbr")
                        nc.gpsimd.partition_broadcast(rbr, rstage)
                        rb = smpool.tile([128, NQ], F32, tag="rb", name="rb")
                        nc.vector.reciprocal_approx_fast(rb, rbr)
                        if h01 == 0:
                            nc.vector.tensor_tensor(
                                ocat_ts[hp][0:64, :], ocopy,
                                rb[0:64, :], mybir.AluOpType.mult)
                        else:
                            s64 = smpool.tile([64, NQ], BF16, tag="s64",
                                              name="s64")
                            nc.vector.tensor_tensor(
                                s64, ocopy, rb[0:64, :],
                                mybir.AluOpType.mult)
                            nc.sync.dma_start(ocat_ts[hp][64:128, :], s64)

                for hp, ti, chunk, is_first, is_last in tiles:
                    if is_first:
                        if hp == 6:
                            for mc in range(8):
                                t = cpool.tile([128, MODEL], BF16,
                                               tag=f"wt{mc}", name=f"wt{mc}")
                                nc.sync.dma_start(t, wt_d[mc])
                                wt_ts.append(t)
                            bv_t = cpool.tile([128, 8], F32, tag="bv",
                                              name="bv")
                            nc.sync.dma_start(bv_t, bv_d[:, :])
                        kt_t = ktpool.tile([128, N], BF16, tag="kt", name="kt")
                        if hp == 0:
                            nc.sync.dma_start(kt_t[:, :256], kt_d[hp][:, :256])
                            nc.sync.dma_start(kt_t[:, 256:], kt_d[hp][:, 256:])
                        else:
                            nc.sync.dma_start(kt_t, kt_d[hp])
                        if hp + 1 < 8:
                            nc.sync.dma_start(qt_ts[hp + 1], qt_d[hp + 1])
                        btu_t = btupool.tile([128, 6, NQ], I16, tag="btu",
                                             name="btu")
                        nc.sync.dma_start(
                            btu_t, btu_d[hp].rearrange("p (s f) -> p s f", s=6))
                        btm_t = btmpool.tile([128, 6, NQ], BF16, tag="btm",
                                             name="btm")
                        nc.sync.dma_start(
                            btm_t, btm_d[hp].rearrange("p (s f) -> p s f", s=6))
                        vv_ts, o_pss = [], []
                        for h01 in range(2):
                            h = 2 * hp + h01
                            vv_t = vvpool.tile([128, CHUNKS, D + 1], BF16,
                                               tag="vv", name="vv")
                            nc.sync.dma_start(vv_t, vv_d[h])
                            vv_ts.append(vv_t)
                            o_pss.append(opool.tile([D + 1, NQ], F32,
                                                    tag="ops", name="ops"))
                        state[hp] = (kt_t, vv_ts, btu_t, btm_t, o_pss)
                    kt_t, vv_ts, btu_t, btm_t, o_pss = state[hp]
                    nu = len(chunk)
                    gw = nu * NQ
                    st = stp.tile([128, 3 * NQ], F32, tag="st", name="st")
                    for uu, (s, h01) in enumerate(chunk):
                        lo, hi = h01 * 64, h01 * 64 + 64
                        nc.tensor.matmul(
                            st[:, uu * NQ:(uu + 1) * NQ],
                            kt_t[lo:hi, s * 128:(s + 1) * 128],
                            qt_ts[hp][lo:hi, :],
                            start=True, stop=True)
                    if hp == 0 and ti in (0, 1, 2):
                        # fill the pipeline-fill PE gaps of the first head-
                        # pair so the HAM activity window stays busy; the
                        # garbage is overwritten by the first real PV's
                        # start=True.
                        for _ in range(3):
                            nc.tensor.matmul(
                                o_pss[0], wrm[0:64, 0:65],
                                qt_ts[hp][0:64, :],
                                start=True, stop=True)
                    u0 = 3 * ti
                    if ti in DVE_TILES:
                        ptile = pmpool.tile([128, 3 * NQ], BF16, tag="pm",
                                            name="pm")
                        if ti == 5:
                            # const-only tile: plain Schraudolph
                            nc.vector.tensor_scalar(
                                ptile[:, :gw].bitcast(I16), st[:, :gw],
                                1.0, BBITS,
                                mybir.AluOpType.mult, mybir.AluOpType.add)
                        else:
                            k0 = 0 if ti == 7 else 3
                            nc.vector.scalar_tensor_tensor(
                                ptile[:, :gw].bitcast(I16), st[:, :gw], 0.0,
                                btu_t[:, k0:k0 + nu, :].rearrange(
                                    "p a b -> p (a b)"),
                                mybir.AluOpType.add, mybir.AluOpType.add)
                        srcs = [ptile[:, uu * NQ:(uu + 1) * NQ]
                                for uu in range(nu)]
                    else:
                        ptile = p0pool.tile([128, 3 * NQ], BF16, tag="p0",
                                            name="p0")
                        nc.scalar.activation(
                            ptile[:, :gw], st[:, :gw],
                            mybir.ActivationFunctionType.Exp,
                            bias=0.0, scale=EXP_SCALE)
                        srcs = [ptile[:, uu * NQ:(uu + 1) * NQ]
                                for uu in range(nu)]
                        # band units on ACT tiles: multiply in the bf16
                        # exp(C*bias) factor; adjacent units share one op.
                        runs = []   # (uu0, n, btm_k0)
                        for uu, (s, h01) in enumerate(chunk):
                            u = u0 + uu
                            if u in BTM_UNITS:
                                k = BTM_UNITS.index(u)
                                if runs and runs[-1][0] + runs[-1][1] == uu \
                                        and runs[-1][2] + runs[-1][1] == k:
                                    runs[-1][1] += 1
                                else:
                                    runs.append([uu, 1, k])
                        for uu0, n, k in runs:
                            pm = pmpool.tile([128, 3 * NQ], BF16, tag="pm",
                                             name="pm")
                            nc.vector.tensor_tensor(
                                pm[:, :n * NQ],
                                ptile[:, uu0 * NQ:(uu0 + n) * NQ],
                                btm_t[:, k:k + n, :].rearrange(
                                    "p a b -> p (a b)"),
                                mybir.AluOpType.mult)
                            for j in range(n):
                                srcs[uu0 + j] = pm[:, j * NQ:(j + 1) * NQ]
                    pv_queue.append((hp, chunk, srcs, is_last))
                    # pop PV work in 2-tile batches: the PE pays ~120ns per
                    # QK<->PV weight/geometry switch, so fewer, larger blocks.
                    tile_count += 1
                    # pop after ODD tile indices: those boundaries fall on
                    # even unit counts, so the injected PV block never splits
                    # an h0/h1 QK row-pair (a split pair runs as two solo
                    # half-width windows: +216ns each).
                    if tile_count % 2 == 1:
                        while len(pv_queue) > DELAY:
                            qhp, qchunk, qsrcs, qlast = pv_queue.pop(0)
                            emit_pv(qhp, qchunk, qsrcs)
                            if qlast:
                                emit_normalize(qhp)
                for qhp, qchunk, qsrcs, qlast in pv_queue:
                    emit_pv(qhp, qchunk, qsrcs)
                    if qlast:
                        emit_normalize(qhp)

            with tc.tile_pool(name="ysb", bufs=2) as ypool, \
                 tc.tile_pool(name="fin", bufs=3, space="PSUM") as fpool:
                for ocp in range(4):
                    fp = fpool.tile([128, 2 * NQ], F32, tag="fp", name="fp")
                    for mc in range(8):
                        for half in range(2):
                            oc = 2 * ocp + half
                            nc.tensor.matmul(
                                fp[:, half * NQ:(half + 1) * NQ],
                                wt_ts[mc][:, oc * 128:(oc + 1) * 128],
                                ocat_ts[mc], start=(mc == 0), stop=(mc == 7))
                    ysb = ypool.tile([128, 2 * NQ], F32, tag="ysb", name="ysb")
                    for half in range(2):
                        oc = 2 * ocp + half
                        nc.scalar.add(ysb[:, half * NQ:(half + 1) * NQ],
                                      fp[:, half * NQ:(half + 1) * NQ],
                                      bv_t[:, oc:oc + 1])
                        nc.sync.dma_start(yt_d[oc],
                                          ysb[:, half * NQ:(half + 1) * NQ])
    nc.compile()
    return nc


def _rel_pos_bucket_np(rel):
    """T5 bidirectional bucketing, float32 math mirroring the jnp reference."""
    nb = NUM_BUCKETS // 2
    ret = (rel >= 0).astype(np.int32) * nb
    n = np.abs(rel)
    max_exact = nb // 2
    is_small = n < max_exact
    n_safe = np.maximum(n, 1).astype(np.float32)
    val_large = max_exact + (
        np.log(n_safe / np.float32(max_exact)).astype(np.float32)
        / np.float32(math.log(MAX_DIST / max_exact)) * np.float32(nb - max_exact)
    ).astype(np.int32)
    val_large = np.minimum(val_large, nb - 1)
    return ret + np.where(is_small, n, val_large)


def _tables(rel_emb):
    """Per-relative-offset tables: int16 Schraudolph bias bits and bf16
    multiplicative exp factors, both [H, 4095] for r in [-2047, 2047]."""
    rel = np.arange(-2047, 2048, dtype=np.int32)
    buckets = _rel_pos_bucket_np(rel)
    bias = np.asarray(rel_emb, np.float32)[buckets, :]          # [4095, H]
    bits = np.round(np.float32(LAM) * bias + np.float32(BBITS)).astype(np.int16)
    fac = np.exp(np.float32(C) * bias).astype(np.float32)
    return np.ascontiguousarray(bits.T), np.ascontiguousarray(fac.T)


def _prep_inputs(q, k, v, rel_emb, w_out, b_out):
    q = np.asarray(q, np.float32)
    k = np.asarray(k, np.float32)
    v = np.asarray(v, np.float32)
    rel_emb = np.asarray(rel_emb, np.float32)
    bits_diag, fac_diag = _tables(rel_emb)
    e_pos = np.exp(np.float32(C) * rel_emb[31, :])   # k - q >= 128
    e_neg = np.exp(np.float32(C) * rel_emb[15, :])   # k - q <= -128
    wt = np.ascontiguousarray(np.asarray(w_out, np.float32).T).reshape(8, 128, MODEL)
    bv = np.ascontiguousarray(np.asarray(b_out, np.float32).reshape(8, 128).T)
    p = np.arange(128)
    u = np.arange(NQ)
    in_maps = []
    for core in range(NCORES):
        b, qc = divmod(core, 4)
        qoff = qc * NQ
        smap = _slot_map(qoff)                       # slot -> chunk j
        qs = (q[b, qoff:qoff + NQ] * np.float32(LAM)).reshape(NQ, 8, 2, 64)
        qt = np.ascontiguousarray(qs.transpose(1, 2, 3, 0)).reshape(8, 128, NQ)
        kt = np.ascontiguousarray(
            k[b].reshape(N, 8, 2, 64).transpose(1, 2, 3, 0)).reshape(8, 128, N)
        kt = np.ascontiguousarray(
            kt.reshape(8, 128, CHUNKS, 128)[:, :, smap, :]).reshape(8, 128, N)
        vs = v[b].reshape(CHUNKS, 128, H, D).transpose(2, 1, 0, 3)  # [h,kk,j,d]
        vv = np.concatenate(
            [vs, np.ones((H, 128, CHUNKS, 1), np.float32)], axis=-1)
        vv = vv[:, :, smap, :]                       # slot order
        for s in range(CHUNKS - NBAND):
            j = smap[s]
            rel_min = 128 * j - qoff - (NQ - 1)      # min over tile of k - q
            rel_max = 128 * j + 127 - qoff
            if rel_min >= 128:
                fac = e_pos
            elif rel_max <= -128:
                fac = e_neg
            else:
                raise AssertionError(
                    f"band chunk {j} in const slot {s} (qoff={qoff})")
            vv[:, :, s, :] *= fac[:, None, None]
        # unit-ordered band tables (unit = 2*slot + h01)
        btu = np.empty((8, 128, 6, NQ), np.int16)
        btm = np.empty((8, 128, 6, NQ), np.float32)
        for arr, unit_list, diag in ((btu, BTU_UNITS, bits_diag),
                                     (btm, BTM_UNITS, fac_diag)):
            for kk, unit in enumerate(unit_list):
                slot, h01 = divmod(unit, 2)
                j = smap[slot]
                idx = (128 * j + p[:, None]) - (qoff + u[None, :]) + 2047
                for hp in range(8):
                    arr[hp, :, kk, :] = diag[2 * hp + h01][idx]
        in_maps.append({
            "qt": qt.astype(ml_dtypes.bfloat16),
            "kt": kt.astype(ml_dtypes.bfloat16),
            "vv": np.ascontiguousarray(vv).astype(ml_dtypes.bfloat16),
            "btu": np.ascontiguousarray(btu.reshape(8, 128, 6 * NQ)),
            "btm": np.ascontiguousarray(
                btm.reshape(8, 128, 6 * NQ).astype(ml_dtypes.bfloat16)),
            "wt": wt.astype(ml_dtypes.bfloat16), "bv": bv,
        })
    return in_maps


def _run(q, k, v, rel_emb, w_out, b_out, trace=False):
    if "nc" not in _CACHE:
        _CACHE["nc"] = _build_bass()
    nc = _CACHE["nc"]
    in_maps = _prep_inputs(q, k, v, rel_emb, w_out, b_out)
    res = run_bass_kernel_spmd(nc, in_maps, core_ids=list(range(NCORES)),
                               trace=trace)
    y = np.empty((B, N, MODEL), np.float32)
    for core in range(NCORES):
        b, qc = divmod(core, 4)
        qoff = qc * NQ
        yt = res.results[core]["yt"]
        y[b, qoff:qoff + NQ] = yt.transpose(2, 0, 1).reshape(NQ, MODEL)
    return y, res


def kernel(q, k, v, rel_emb, w_out, b_out):
    y, _ = _run(q, k, v, rel_emb, w_out, b_out, trace=False)
    return y


# revision 55
# speedup vs baseline: 1.0257x; 1.0061x over previous
"""Trainium2 Bass kernel for T5-style relative-position-bias attention.

Problem (hardcoded): B=2, N=2048, H=16, D=64, MODEL=1024
  sim  = q @ k^T per head; sim = (sim + rel_pos_bias) * D**-0.5
  attn = softmax(sim, axis=-1)
  out  = (attn @ v) reshaped to [b, n, MODEL] @ w_out.T + b_out

Sharding: 8 cores = (batch b) x (query-chunk qoff in {0,512,1024,1536}).
Each core computes the full output rows for its 512 queries; no collectives.

Device algorithm per core (transposes all pushed to host):
  S^T[k, q] = kT_h-slices.T @ qT_h   (bf16 matmuls, contraction d=64).
  Host pre-scales q by LAM = 128*log2(e)*D**-0.5 so PSUM logits arrive in
  "bf16 exponent bits" scale: S = LAM*(q.k).
  Softmax exp is split across TWO engines to beat the single-ACT pace
  (ACT alone paces the pipeline at ~17.3us per head-pair):
   - tiles 0-4,6,8,10 of each head-pair: ACT Exp (scale=ln2/128).
   - tiles 5,7,9: ONE Vector-engine scalar_tensor_tensor per tile -
     int16(S + table) is the Schraudolph bf16-bit-pattern of exp
     (~1.8% rms on those keys only; softmax-averaged ~1.2% output error
     vs the 2e-2 gate).  Tiles 7/9 are all-band: their int16 table also
     carries the exact T5 bias (round(LAM*bias + BBITS)); tile 5 is
     const-only (tensor_scalar, bias folded into V').
    Positions 5,7,9 make the DVE stream chain-affine in the 2-buffer
    PSUM rotation: QK(7) waits ew(5)=DVE, QK(9) waits ew(7)=DVE, so the
    two engines stream their own dependency chains concurrently.
  T5 buckets saturate for |k-q| >= 128: chunks fully off the band have a
  CONSTANT bias factor per head, folded into host-pre-scaled V' tiles.
  Band chunks on ACT tiles (units 20,24,25,26,30,31) instead multiply by
  a bf16 exp(C*bias) table on the Vector engine.
  Key chunks are host-permuted into a fixed slot order (const slots 0..9,
  band slots 10..15) so all 8 cores run one identical program.
  No max-subtraction: logits are ~N(0,1) after scaling (safe in fp32).
  O^T[m, q] = sum_k V'[k, m] * P[k, q], V' = [V | ones] (row 64 = denom r).
  Normalize via broadcast 1/r, then y^T = w_out @ O^T + b_out (host
  re-transposes the per-core [1024, 512] outputs).
  Dummy warmup matmuls + a dummy Exp during the initial DMA window
  pre-warm the HAM clock gate and pre-load the ACT exp table.
"""
import sys
import math

sys.path.insert(0, "/opt/trn_rl_repo")

import numpy as np
import ml_dtypes

import concourse.bass as bass
from concourse import bacc
import concourse.tile as tile
from concourse import mybir
from concourse.bass_utils import run_bass_kernel_spmd

F32 = mybir.dt.float32
I16 = mybir.dt.int16
BF16 = mybir.dt.bfloat16

B, N, H, D = 2, 2048, 16, 64
MODEL = H * D
NQ = 512
NCORES = 8
C = float(D) ** -0.5
NUM_BUCKETS, MAX_DIST = 32, 128
CHUNKS = 16
NBAND = 6                       # band slots 10..15
LAM = 128.0 * C * math.log2(math.e)   # logits -> bf16-bit scale
BBITS = 16248.75                # Schraudolph bias (HW rounds f32->i16)
EXP_SCALE = math.log(2.0) / 128.0     # exp(S*EXP_SCALE) == exp(C*s)

DVE_TILES = (5, 7, 9)           # tiles exp'd on the Vector engine
# unit u = 2*slot + h01; tiles of 3 units; band units are 20..31
BTU_UNITS = (21, 22, 23, 27, 28, 29)  # DVE-side band units (int16 bias table)
BTM_UNITS = (20, 24, 25, 26, 30, 31)  # ACT-side band units (bf16 factor table)

_CACHE = {}


def _slot_map(qoff):
    """Permutation slot -> original key-chunk j. Band chunks (those touching
    |k-q| < 128 for q in [qoff, qoff+512)) go to slots 10..15; if fewer than
    6 band chunks exist, nearest const chunks fill the extra band slots."""
    j_lo = max(0, -(-(qoff - 254) // 128))          # ceil((qoff-254)/128)
    j_hi = min(CHUNKS - 1, (qoff + 638) // 128)     # floor
    band = list(range(j_lo, j_hi + 1))
    while len(band) < NBAND:                        # pad with neighbors
        if band[0] > 0:
            band.insert(0, band[0] - 1)
        else:
            band.append(band[-1] + 1)
    const = [j for j in range(CHUNKS) if j not in band]
    assert len(band) == NBAND and len(const) == CHUNKS - NBAND
    return const + band                             # slots 0..9 const, 10..15 band


def _build_bass():
    nc = bacc.Bacc("TRN2", target_bir_lowering=False, debug=False,
                   num_devices=NCORES)
    qt_d = nc.dram_tensor("qt", [8, 128, NQ], BF16, kind="ExternalInput")
    kt_d = nc.dram_tensor("kt", [8, 128, N], BF16, kind="ExternalInput")
    vv_d = nc.dram_tensor("vv", [H, 128, CHUNKS, D + 1], BF16, kind="ExternalInput")
    btu_d = nc.dram_tensor("btu", [8, 128, 6 * NQ], I16, kind="ExternalInput")
    btm_d = nc.dram_tensor("btm", [8, 128, 6 * NQ], BF16, kind="ExternalInput")
    wt_d = nc.dram_tensor("wt", [8, 128, MODEL], BF16, kind="ExternalInput")
    bv_d = nc.dram_tensor("bv", [128, 8], F32, kind="ExternalInput")
    yt_d = nc.dram_tensor("yt", [8, 128, NQ], F32, kind="ExternalOutput")

    with tile.TileContext(nc) as tc:
        with tc.tile_pool(name="const", bufs=1) as cpool:
            qt_ts = []
            for hp in range(8):
                t = cpool.tile([128, NQ], BF16, tag=f"qt{hp}", name=f"qt{hp}")
                qt_ts.append(t)
            nc.sync.dma_start(qt_ts[0], qt_d[0])
            # Warmup during the input-DMA window: dummy matmuls pre-warm the
            # HAM clock gate; a dummy Exp pre-loads the ACT spline table.
            wrm = cpool.tile([128, 128], BF16, tag="wrm", name="wrm")
            nc.vector.memset(wrm, 0.0)
            wrm_e = cpool.tile([1, 64], F32, tag="wrme", name="wrme")
            nc.scalar.activation(wrm_e, wrm[0:1, :64],
                                 mybir.ActivationFunctionType.Exp,
                                 bias=0.0, scale=0.0)
            with tc.tile_pool(name="wrmps", bufs=1, space="PSUM") as wpsp:
                wps = wpsp.tile([128, 128], F32, tag="wps", name="wps")
                for _ in range(30):
                    nc.tensor.matmul(wps, wrm, wrm, start=True, stop=True)
            wt_ts = []
            bv_t = None
            ocat_ts = [cpool.tile([128, NQ], BF16, tag=f"ocat{mc}",
                                  name=f"ocat{mc}")
                       for mc in range(8)]

            with tc.tile_pool(name="kt", bufs=2) as ktpool, \
                 tc.tile_pool(name="vv", bufs=4) as vvpool, \
                 tc.tile_pool(name="btu", bufs=2) as btupool, \
                 tc.tile_pool(name="btm", bufs=2) as btmpool, \
                 tc.tile_pool(name="p0", bufs=8) as p0pool, \
                 tc.tile_pool(name="pm", bufs=12) as pmpool, \
                 tc.tile_pool(name="sm", bufs=6) as smpool, \
                 tc.tile_pool(name="stps", bufs=2, space="PSUM") as stp, \
                 tc.tile_pool(name="ops", bufs=2, space="PSUM") as opool:
                DELAY = 4
                units_per_pair = [(s, h01) for s in range(CHUNKS)
                                  for h01 in range(2)]
                tiles = []
                for hp in range(8):
                    for ti, u0 in enumerate(range(0, 32, 3)):
                        tiles.append((hp, ti, units_per_pair[u0:u0 + 3],
                                      u0 == 0, u0 + 3 >= 32))
                state = {}      # hp -> (kt_t, vv_ts, btu_t, btm_t, o_pss)
                pv_queue = []   # (hp, chunk, srcs)
                tile_count = 0

                def emit_pv(hp, chunk, srcs):
                    _, vv_ts, _, _, o_pss = state[hp]
                    for uu, (s, h01) in enumerate(chunk):
                        nc.tensor.matmul(
                            o_pss[h01], vv_ts[h01][:, s, :], srcs[uu],
                            start=(s == 0), stop=(s == CHUNKS - 1))

                def emit_normalize(hp):
                    o_pss = state[hp][4]
                    for h01 in range(2):
                        o_ps = o_pss[h01]
                        rstage = smpool.tile([1, NQ], F32, tag="rstage",
                                             name="rstage")
                        nc.vector.tensor_copy(rstage[0:1, :], o_ps[64:65, :])
                        ocopy = smpool.tile([64, NQ], F32, tag="ocopy",
                                            name="ocopy")
                        nc.vector.tensor_copy(ocopy, o_ps[0:64, :])
                        rbr = smpool.tile([128, NQ], F32, tag="rbr",
                                          name="rbr")
                        nc.gpsimd.partition_broadcast(rbr, rstage)
                        rb = smpool.tile([128, NQ], F32, tag="rb", name="rb")
                        nc.vector.reciprocal_approx_fast(rb, rbr)
                        if h01 == 0:
                            nc.vector.tensor_tensor(
                                ocat_ts[hp][0:64, :], ocopy,
                                rb[0:64, :], mybir.AluOpType.mult)
                        else:
                            s64 = smpool.tile([64, NQ], BF16, tag="s64",
                                              name="s64")
                            nc.vector.tensor_tensor(
                                s64, ocopy, rb[0:64, :],
                                mybir.AluOpType.mult)
                            nc.sync.dma_start(ocat_ts[hp][64:128, :], s64)

                for hp, ti, chunk, is_first, is_last in tiles:
                    if is_first:
                        if hp == 6:
                            for mc in range(8):
                                t = cpool.tile([128, MODEL], BF16,
                                               tag=f"wt{mc}", name=f"wt{mc}")
                                nc.sync.dma_start(t, wt_d[mc])
                                wt_ts.append(t)
                            bv_t = cpool.tile([128, 8], F32, tag="bv",
                                              name="bv")
                            nc.sync.dma_start(bv_t, bv_d[:, :])
                        kt_t = ktpool.tile([128, N], BF16, tag="kt", name="kt")
                        if hp == 0:
                            nc.sync.dma_start(kt_t[:, :256], kt_d[hp][:, :256])
                            nc.sync.dma_start(kt_t[:, 256:], kt_d[hp][:, 256:])
                        else:
                            nc.sync.dma_start(kt_t, kt_d[hp])
                        if hp + 1 < 8:
                            nc.sync.dma_start(qt_ts[hp + 1], qt_d[hp + 1])
                        btu_t = btupool.tile([128, 6, NQ], I16, tag="btu",
                                             name="btu")
                        nc.sync.dma_start(
                            btu_t, btu_d[hp].rearrange("p (s f) -> p s f", s=6))
                        btm_t = btmpool.tile([128, 6, NQ], BF16, tag="btm",
                                             name="btm")
                        nc.sync.dma_start(
                            btm_t, btm_d[hp].rearrange("p (s f) -> p s f", s=6))
                        vv_ts, o_pss = [], []
                        for h01 in range(2):
                            h = 2 * hp + h01
                            vv_t = vvpool.tile([128, CHUNKS, D + 1], BF16,
                                               tag="vv", name="vv")
                            nc.sync.dma_start(vv_t, vv_d[h])
                            vv_ts.append(vv_t)
                            o_pss.append(opool.tile([D + 1, NQ], F32,
                                                    tag="ops", name="ops"))
                        state[hp] = (kt_t, vv_ts, btu_t, btm_t, o_pss)
                    kt_t, vv_ts, btu_t, btm_t, o_pss = state[hp]
                    nu = len(chunk)
                    gw = nu * NQ
                    st = stp.tile([128, 3 * NQ], F32, tag="st", name="st")
                    for uu, (s, h01) in enumerate(chunk):
                        lo, hi = h01 * 64, h01 * 64 + 64
                        nc.tensor.matmul(
                            st[:, uu * NQ:(uu + 1) * NQ],
                            kt_t[lo:hi, s * 128:(s + 1) * 128],
                            qt_ts[hp][lo:hi, :],
                            start=True, stop=True)
                    if hp == 0 and ti in (0, 1, 2):
                        # fill the pipeline-fill PE gaps of the first head-
                        # pair so the HAM activity window stays busy; the
                        # garbage is overwritten by the first real PV's
                        # start=True.
                        for _ in range(3):
                            nc.tensor.matmul(
                                o_pss[0], wrm[0:64, 0:65],
                                qt_ts[hp][0:64, :],
                                start=True, stop=True)
                    u0 = 3 * ti
                    if ti in DVE_TILES:
                        ptile = pmpool.tile([128, 3 * NQ], BF16, tag="pm",
                                            name="pm")
                        if ti == 5:
                            # const-only tile: plain Schraudolph
                            nc.vector.tensor_scalar(
                                ptile[:, :gw].bitcast(I16), st[:, :gw],
                                1.0, BBITS,
                                mybir.AluOpType.mult, mybir.AluOpType.add)
                        else:
                            k0 = 0 if ti == 7 else 3
                            nc.vector.scalar_tensor_tensor(
                                ptile[:, :gw].bitcast(I16), st[:, :gw], 0.0,
                                btu_t[:, k0:k0 + nu, :].rearrange(
                                    "p a b -> p (a b)"),
                                mybir.AluOpType.add, mybir.AluOpType.add)
                        srcs = [ptile[:, uu * NQ:(uu + 1) * NQ]
                                for uu in range(nu)]
                    else:
                        ptile = p0pool.tile([128, 3 * NQ], BF16, tag="p0",
                                            name="p0")
                        nc.scalar.activation(
                            ptile[:, :gw], st[:, :gw],
                            mybir.ActivationFunctionType.Exp,
                            bias=0.0, scale=EXP_SCALE)
                        srcs = [ptile[:, uu * NQ:(uu + 1) * NQ]
                                for uu in range(nu)]
                        # band units on ACT tiles: multiply in the bf16
                        # exp(C*bias) factor; adjacent units share one op.
                        runs = []   # (uu0, n, btm_k0)
                        for uu, (s, h01) in enumerate(chunk):
                            u = u0 + uu
                            if u in BTM_UNITS:
                                k = BTM_UNITS.index(u)
                                if runs and runs[-1][0] + runs[-1][1] == uu \
                                        and runs[-1][2] + runs[-1][1] == k:
                                    runs[-1][1] += 1
                                else:
                                    runs.append([uu, 1, k])
                        for uu0, n, k in runs:
                            pm = pmpool.tile([128, 3 * NQ], BF16, tag="pm",
                                             name="pm")
                            nc.vector.tensor_tensor(
                                pm[:, :n * NQ],
                                ptile[:, uu0 * NQ:(uu0 + n) * NQ],
                                btm_t[:, k:k + n, :].rearrange(
                                    "p a b -> p (a b)"),
                                mybir.AluOpType.mult)
                            for j in range(n):
                                srcs[uu0 + j] = pm[:, j * NQ:(j + 1) * NQ]
                    pv_queue.append((hp, chunk, srcs, is_last))
                    # pop PV work in 2-tile batches: the PE pays ~120ns per
                    # QK<->PV weight/geometry switch, so fewer, larger blocks.
                    tile_count += 1
                    # pop after ODD tile indices: those boundaries fall on
                    # even unit counts, so the injected PV block never splits
                    # an h0/h1 QK row-pair (a split pair runs as two solo
                    # half-width windows: +216ns each).
                    if tile_count % 2 == 1:
                        while len(pv_queue) > DELAY:
                            qhp, qchunk, qsrcs, qlast = pv_queue.pop(0)
                            emit_pv(qhp, qchunk, qsrcs)
                            if qlast:
                                emit_normalize(qhp)
                for qhp, qchunk, qsrcs, qlast in pv_queue:
                    emit_pv(qhp, qchunk, qsrcs)
                    if qlast:
                        emit_normalize(qhp)

            with tc.tile_pool(name="ysb", bufs=2) as ypool, \
                 tc.tile_pool(name="fin", bufs=3, space="PSUM") as fpool:
                for ocp in range(4):
                    fp = fpool.tile([128, 2 * NQ], F32, tag="fp", name="fp")
                    for mc in range(8):
                        for half in range(2):
                            oc = 2 * ocp + half
                            nc.tensor.matmul(
                                fp[:, half * NQ:(half + 1) * NQ],
                                wt_ts[mc][:, oc * 128:(oc + 1) * 128],
                                ocat_ts[mc], start=(mc == 0), stop=(mc == 7))
                    ysb = ypool.tile([128, 2 * NQ], F32, tag="ysb", name="ysb")
                    for half in range(2):
                        oc = 2 * ocp + half
                        nc.scalar.add(ysb[:, half * NQ:(half + 1) * NQ],
                                      fp[:, half * NQ:(half + 1) * NQ],
                                      bv_t[:, oc:oc + 1])
                        nc.sync.dma_start(yt_d[oc],
                                          ysb[:, half * NQ:(half + 1) * NQ])
    nc.compile()
    return nc


def _rel_pos_bucket_np(rel):
    """T5 bidirectional bucketing, float32 math mirroring the jnp reference."""
    nb = NUM_BUCKETS // 2
    ret = (rel >= 0).astype(np.int32) * nb
    n = np.abs(rel)
    max_exact = nb // 2
    is_small = n < max_exact
    n_safe = np.maximum(n, 1).astype(np.float32)
    val_large = max_exact + (
        np.log(n_safe / np.float32(max_exact)).astype(np.float32)
        / np.float32(math.log(MAX_DIST / max_exact)) * np.float32(nb - max_exact)
    ).astype(np.int32)
    val_large = np.minimum(val_large, nb - 1)
    return ret + np.where(is_small, n, val_large)


def _tables(rel_emb):
    """Per-relative-offset tables: int16 Schraudolph bias bits and bf16
    multiplicative exp factors, both [H, 4095] for r in [-2047, 2047]."""
    rel = np.arange(-2047, 2048, dtype=np.int32)
    buckets = _rel_pos_bucket_np(rel)
    bias = np.asarray(rel_emb, np.float32)[buckets, :]          # [4095, H]
    bits = np.round(np.float32(LAM) * bias + np.float32(BBITS)).astype(np.int16)
    fac = np.exp(np.float32(C) * bias).astype(np.float32)
    return np.ascontiguousarray(bits.T), np.ascontiguousarray(fac.T)


def _prep_inputs(q, k, v, rel_emb, w_out, b_out):
    q = np.asarray(q, np.float32)
    k = np.asarray(k, np.float32)
    v = np.asarray(v, np.float32)
    rel_emb = np.asarray(rel_emb, np.float32)
    bits_diag, fac_diag = _tables(rel_emb)
    e_pos = np.exp(np.float32(C) * rel_emb[31, :])   # k - q >= 128
    e_neg = np.exp(np.float32(C) * rel_emb[15, :])   # k - q <= -128
    wt = np.ascontiguousarray(np.asarray(w_out, np.float32).T).reshape(8, 128, MODEL)
    bv = np.ascontiguousarray(np.asarray(b_out, np.float32).reshape(8, 128).T)
    p = np.arange(128)
    u = np.arange(NQ)
    in_maps = []
    for core in range(NCORES):
        b, qc = divmod(core, 4)
        qoff = qc * NQ
        smap = _slot_map(qoff)                       # slot -> chunk j
        qs = (q[b, qoff:qoff + NQ] * np.float32(LAM)).reshape(NQ, 8, 2, 64)
        qt = np.ascontiguousarray(qs.transpose(1, 2, 3, 0)).reshape(8, 128, NQ)
        kt = np.ascontiguousarray(
            k[b].reshape(N, 8, 2, 64).transpose(1, 2, 3, 0)).reshape(8, 128, N)
        kt = np.ascontiguousarray(
            kt.reshape(8, 128, CHUNKS, 128)[:, :, smap, :]).reshape(8, 128, N)
        vs = v[b].reshape(CHUNKS, 128, H, D).transpose(2, 1, 0, 3)  # [h,kk,j,d]
        vv = np.concatenate(
            [vs, np.ones((H, 128, CHUNKS, 1), np.float32)], axis=-1)
        vv = vv[:, :, smap, :]                       # slot order
        for s in range(CHUNKS - NBAND):
            j = smap[s]
            rel_min = 128 * j - qoff - (NQ - 1)      # min over tile of k - q
            rel_max = 128 * j + 127 - qoff
            if rel_min >= 128:
                fac = e_pos
            elif rel_max <= -128:
                fac = e_neg
            else:
                raise AssertionError(
                    f"band chunk {j} in const slot {s} (qoff={qoff})")
            vv[:, :, s, :] *= fac[:, None, None]
        # unit-ordered band tables (unit = 2*slot + h01)
        btu = np.empty((8, 128, 6, NQ), np.int16)
        btm = np.empty((8, 128, 6, NQ), np.float32)
        for arr, unit_list, diag in ((btu, BTU_UNITS, bits_diag),
                                     (btm, BTM_UNITS, fac_diag)):
            for kk, unit in enumerate(unit_list):
                slot, h01 = divmod(unit, 2)
                j = smap[slot]
                idx = (128 * j + p[:, None]) - (qoff + u[None, :]) + 2047
                for hp in range(8):
                    arr[hp, :, kk, :] = diag[2 * hp + h01][idx]
        in_maps.append({
            "qt": qt.astype(ml_dtypes.bfloat16),
            "kt": kt.astype(ml_dtypes.bfloat16),
            "vv": np.ascontiguousarray(vv).astype(ml_dtypes.bfloat16),
            "btu": np.ascontiguousarray(btu.reshape(8, 128, 6 * NQ)),
            "btm": np.ascontiguousarray(
                btm.reshape(8, 128, 6 * NQ).astype(ml_dtypes.bfloat16)),
            "wt": wt.astype(ml_dtypes.bfloat16), "bv": bv,
        })
    return in_maps


def _run(q, k, v, rel_emb, w_out, b_out, trace=False):
    if "nc" not in _CACHE:
        _CACHE["nc"] = _build_bass()
    nc = _CACHE["nc"]
    in_maps = _prep_inputs(q, k, v, rel_emb, w_out, b_out)
    res = run_bass_kernel_spmd(nc, in_maps, core_ids=list(range(NCORES)),
                               trace=trace)
    y = np.empty((B, N, MODEL), np.float32)
    for core in range(NCORES):
        b, qc = divmod(core, 4)
        qoff = qc * NQ
        yt = res.results[core]["yt"]
        y[b, qoff:qoff + NQ] = yt.transpose(2, 0, 1).reshape(NQ, MODEL)
    return y, res


def kernel(q, k, v, rel_emb, w_out, b_out):
    y, _ = _run(q, k, v, rel_emb, w_out, b_out, trace=False)
    return y
